# revision 25
# baseline (speedup 1.0000x reference)
"""Expert-choice MoE kernel for 8 Trainium2 NeuronCores (Bacc/Tile).

Distribution: expert-parallel, one expert per core.
  - gate: each core computes fp32 scores z = x_shard @ Wg for its 1/8 token
    shard, AllToAll -> each core holds the full (N,) score row of ITS expert.
  - top-k (k=2048 of N=8192): exact threshold via fp32 bisection on counts,
    then index compaction with the gpsimd sparse_gather ucode kernel.
  - dispatch: ONE dma_gather(transpose=True) per 512-token group pulls the
    selected rows from HBM already transposed to [h, tok] bf16 layout.
  - expert FFN in bf16 (fp32 accumulation), erf-Gelu on the scalar engine,
    fp32 gate multiply fused into the bf16 output cast.
  - combine: ONE dma_scatter_add (SDMA CCE add) per group into a zeroed
    bf16 (N, H) dense buffer, ReduceScatter (add, bf16) across the 8 cores,
    upcast the local shard to fp32.
"""

import sys

for _p in ("/opt/trn_rl_repo",):
    if _p not in sys.path:
        sys.path.insert(0, _p)

import numpy as np
import ml_dtypes

import concourse.bass as bass
import concourse.bacc as bacc
import concourse.mybir as mybir
import concourse.tile as tile
from concourse.bass import _add_dep_helper

# ---------------------------------------------------------------------------
# Patch: this walrus build rejects >1 sync-wait on the SP Drain that
# TileContext emits at kernel exit. Split the global-clock waits across
# several drains (1 wait each).
# ---------------------------------------------------------------------------
from concourse.vector_clock import ScopedClock

_MAX_DRAIN_WAITS = 1


def _patched_drain_and_barrier(self, tick_clock, wait_clock):
    nc = self.nc
    probe = nc.sync.drain()
    wait_clock.add_sem_waits(probe.ins, ScopedClock({None: tick_clock.global_clock}))
    si = probe.ins.sync_info
    waits = list(si.on_wait or []) if si is not None else []
    if len(waits) > _MAX_DRAIN_WAITS:
        probe.ins.sync_info = mybir.SyncInfo(
            on_wait=waits[:_MAX_DRAIN_WAITS],
            on_update=list(si.on_update or []),
        )
        for i in range(_MAX_DRAIN_WAITS, len(waits), _MAX_DRAIN_WAITS):
            extra = nc.sync.drain()
            extra.ins.sync_info = mybir.SyncInfo(
                on_wait=waits[i : i + _MAX_DRAIN_WAITS], on_update=[]
            )
    nc.all_engine_barrier()
    assert self.sems is not None
    popped = nc._tile_sem_poison_stack.pop()
    assert popped is self._sem_poison
    nc.clear_and_free_semaphores(list(self.sems.allocated().values()))
    nc.all_engine_barrier()


tile.TileContext._drain_and_barrier = _patched_drain_and_barrier

_WSPLIT_LIMIT = 1
_wsplit_ctr = [0]


def _split_excess_waits(nc, limit=_WSPLIT_LIMIT):
    """This walrus build encodes at most `limit` sync-wait commands per
    instruction; hoist excess waits onto same-engine Drain instructions
    inserted immediately before (per-engine streams execute in order)."""
    f = nc.m.functions[0]
    for b in f.blocks:
        insts = b.instructions
        out = []
        changed = False
        for inst in insts:
            si = getattr(inst, "sync_info", None)
            waits = list(si.on_wait or []) if si is not None else []
            eng = getattr(inst, "engine", None)
            if len(waits) > limit and eng is not None and \
                    eng != mybir.EngineType.Unassigned:
                keep = waits[-limit:]
                extra = waits[:-limit]
                for i in range(0, len(extra), limit):
                    d = mybir.InstDrain(
                        name=f"WSPLIT-{_wsplit_ctr[0]}", ins=[], outs=[])
                    _wsplit_ctr[0] += 1
                    d.engine = eng
                    d.sync_info = mybir.SyncInfo(
                        on_wait=extra[i:i + limit], on_update=[])
                    out.append(d)
                    nc.register_instruction(d, overwrite=True)
                inst.sync_info = mybir.SyncInfo(
                    on_wait=keep, on_update=list(si.on_update or []))
                changed = True
            out.append(inst)
        if changed:
            b.instructions = out

dt = mybir.dt
Alu = mybir.AluOpType
Act = mybir.ActivationFunctionType

N_CORES = 8

FULL = dict(N=8192, H=1024, FF=4096, E=8, K=2048)


def build_moe_nc(N=8192, H=1024, FF=4096, E=8, K=2048, TOKG=512, act=None,
                 do_compile=True, bisect_iters=33, skip_sg=False,
                 skip_ffn=False, skip_rs=False, skip_gather=False,
                 skip_scatter=False):
    """Build the SPMD Bacc program (same program on all 8 cores)."""
    assert E == N_CORES
    P = N // N_CORES          # tokens per shard
    HC = H // 128             # h chunks
    FC = FF // 128            # ff chunks
    NG = K // TOKG            # token groups
    SUBS = TOKG // 128        # 128-token subtiles per group
    NCOLS = K // 128          # compact cols in [128, NCOLS] layout
    ZF = N // 128             # free size of the [128, ZF] score layout
    W16 = N // 16             # free size of the [16, W16] wrapped layout
    K16 = K // 16             # compact cols in [16, K16] wrapped layout
    GCOLS = TOKG // 16        # idx cols consumed per group
    assert K % TOKG == 0 and TOKG % 128 == 0 and P % 128 == 0
    assert K16 <= 512  # sparse_gather output limit
    if act is None:
        act = Act.Gelu
    NSTEP = min(512, H)

    nc = bacc.Bacc(None, target_bir_lowering=False, debug=False,
                   num_devices=N_CORES)

    # ---- I/O ----
    xT_s = nc.dram_tensor("xT_s", [H, P], dt.float32, kind="ExternalInput")
    x_bf = nc.dram_tensor("x_bf", [N, H], dt.bfloat16, kind="ExternalInput")
    Wg_d = nc.dram_tensor("Wg", [H, E], dt.float32, kind="ExternalInput")
    W1_d = nc.dram_tensor("W1", [H, FF], dt.bfloat16, kind="ExternalInput")
    W2_d = nc.dram_tensor("W2", [FF, H], dt.bfloat16, kind="ExternalInput")
    b1_d = nc.dram_tensor("b1", [1, FF], dt.float32, kind="ExternalInput")
    b2_d = nc.dram_tensor("b2", [1, H], dt.float32, kind="ExternalInput")
    # y is emitted in bf16 (the combine is bf16 anyway); the host upcasts
    y_d = nc.dram_tensor("y", [P, H], dt.bfloat16, kind="ExternalOutput")

    # ---- internal DRAM ----
    z_loc_d = nc.dram_tensor("z_loc", [E, P], dt.float32)
    z_e_d = nc.dram_tensor("z_e", [N_CORES, P], dt.float32)
    g_dram = nc.dram_tensor("g_dram", [K], dt.float32)
    dense_d = nc.dram_tensor("dense", [N, H], dt.bfloat16)
    rs_out_d = nc.dram_tensor("rs_out", [P, H], dt.bfloat16)

    groups = [list(range(N_CORES))]

    with tile.TileContext(nc) as tc:
        with (
            tc.tile_pool(name="const", bufs=1) as const_pool,
            tc.tile_pool(name="w", bufs=1) as w_pool,
            tc.tile_pool(name="psum1", bufs=2, space="PSUM") as psum1_pool,
            tc.tile_pool(name="psum2", bufs=2, space="PSUM") as psum2_pool,
        ):
            # ---------------- persistent constants ----------------
            ones1 = const_pool.tile([1, 128], dt.float32)
            nc.vector.memset(ones1[:], 1.0)
            ones128 = const_pool.tile([128, 128], dt.float32)
            nc.vector.memset(ones128[:], 1.0)

            # b2 broadcast [128, H] (constant along tokens)
            b2_sb = const_pool.tile([1, H], dt.float32)
            nc.sync.dma_start(b2_sb[:], b2_d[:])
            b2_ps = psum2_pool.tile([128, H], dt.float32, tag="ps2")
            for hh in range(0, H, NSTEP):
                nc.tensor.matmul(b2_ps[:, hh:hh + NSTEP], ones1[:],
                                 b2_sb[:, hh:hh + NSTEP], start=True, stop=True)
            b2_bcast = const_pool.tile([128, H], dt.float32)
            nc.vector.tensor_copy(b2_bcast[:], b2_ps[:])

            # b1 per-partition [128, FC]
            b1_pp = const_pool.tile([128, FC], dt.float32)
            nc.sync.dma_start(
                b1_pp[:], b1_d[:].rearrange("o (c p) -> (o p) c", p=128))

            # persistent routing outputs (filled by the gate phase)
            idxs_tok = const_pool.tile([128, K16], dt.int16)
            g_pp = const_pool.tile([128, NCOLS], dt.float32)

            # ================= gate phase (scoped pool) ================
            # Emitted BEFORE the (much larger) weight/zero-fill DMAs so the
            # scheduler gives the latency-critical gate inputs DMA priority.
            sg2_inst = None
            with (
                tc.tile_pool(name="gate", bufs=1) as gate_pool,
                tc.tile_pool(name="small", bufs=2) as small_pool,
            ):
                xT_sb = gate_pool.tile([128, HC, P], dt.float32)
                nc.sync.dma_start(
                    xT_sb[:], xT_s[:].rearrange("(c p) t -> p c t", p=128))
                wg_sb = gate_pool.tile([128, HC, E], dt.float32)
                nc.sync.dma_start(
                    wg_sb[:], Wg_d[:].rearrange("(c p) e -> p c e", p=128))

                # z_sb_loc rows are written PRE-SWIZZLED (token u stored at
                # column (u%16)*64 + u//16) so that after the AllToAll the
                # wrapped-16 [16, W16] view is a contiguous-stride load.
                z_sb_loc = gate_pool.tile([E, P], dt.float32)
                z_loc_sw = z_sb_loc[:].rearrange("e (r w) -> e w r", r=16)
                for t0 in range(0, P, 512):
                    zw = min(512, P - t0)
                    z_ps = psum1_pool.tile([E, 512], dt.float32, tag="ps1")
                    for ci in range(HC):
                        nc.tensor.matmul(z_ps[:, :zw], wg_sb[:, ci, :],
                                         xT_sb[:, ci, t0:t0 + zw],
                                         start=(ci == 0), stop=(ci == HC - 1))
                    nc.vector.tensor_copy(
                        z_loc_sw[:, t0 // 16:(t0 + zw) // 16, :],
                        z_ps[:, :zw])
                nc.sync.dma_start(z_loc_d[:], z_sb_loc[:])

                # core c receives every shard's scores for expert c
                nc.gpsimd.collective_compute(
                    "AllToAll", Alu.bypass, replica_groups=groups,
                    ins=[z_loc_d[:]], outs=[z_e_d[:]],
                )

                # wrapped-16 view (token j at [j%16, j//16]); contiguous
                # 64-element runs thanks to the sender-side swizzle
                # (slot shared with the now-dead z_sb_loc)
                z16 = gate_pool.tile([16, W16], dt.float32, tag="z_sb_loc")
                nc.sync.dma_start(
                    z16[:].rearrange("r (q w) -> r q w", q=E),
                    z_e_d[:].rearrange("q (r w) -> r q w", r=16))

                # compaction inputs that do not depend on the threshold —
                # emitted first so they overlap the A2A / bisection
                ids16 = gate_pool.tile([16, W16], dt.int32)
                nc.gpsimd.iota(ids16[:], pattern=[[16, W16]], base=0,
                               channel_multiplier=1)
                idf16 = gate_pool.tile([16, W16], dt.float32)
                nc.vector.tensor_copy(idf16[:], ids16[:])
                sig16 = gate_pool.tile([16, W16], dt.float32)
                nc.scalar.activation(sig16[:], z16[:], Act.Sigmoid)


                # ---- parallel-128 search for the k-th largest score ----
                # Every partition holds ALL N scores (PE row-broadcast of
                # z16); round r tests the 128 candidates base + p*step_r at
                # once (one tensor_scalar with accum_out gives, for every
                # partition p, the count of scores >= its candidate).
                # base' = base + (j*-1)*step with j* = #candidates whose
                # count >= K keeps count(z >= base) >= K invariant; each
                # round shrinks the bracket 128x. 5 rounds -> 3.7e-9, below
                # fp32 ulp of the scores.
                # eqm[k', k*128+m] = (k==k'): selector for the broadcast
                # int iota shares the (later-used) scr8 slot — same 8KB
                eqm_i = gate_pool.tile([16, 16 * 128], dt.int32, tag="scr8")
                nc.gpsimd.iota(eqm_i[:], pattern=[[1, 16], [0, 128]], base=0,
                               channel_multiplier=-1)
                eqm = gate_pool.tile([16, 16 * 128], dt.float32)
                nc.vector.tensor_scalar(eqm[:], eqm_i[:], 0, None,
                                        op0=Alu.is_equal)
                iota_p = gate_pool.tile([128, 1], dt.int32)
                nc.gpsimd.iota(iota_p[:], pattern=[[1, 1]], base=0,
                               channel_multiplier=1)
                iota_pf = gate_pool.tile([128, 1], dt.float32)
                nc.vector.tensor_copy(iota_pf[:], iota_p[:])

                z_bcast = gate_pool.tile([128, 16, W16], dt.float32,
                                         tag="xT_sb")  # reuse the xT slot
                for k in range(16):
                    for w0 in range(0, W16, 512):
                        ww = min(512, W16 - w0)
                        zb_ps = psum1_pool.tile([128, 512], dt.float32,
                                                tag="ps1")
                        nc.tensor.matmul(zb_ps[:, :ww],
                                         eqm[:, k * 128:(k + 1) * 128],
                                         z16[:, w0:w0 + ww],
                                         start=True, stop=True)
                        nc.vector.tensor_copy(z_bcast[:, k, w0:w0 + ww],
                                              zb_ps[:, :ww])

                kf = float(K)
                rounds = max(1, min(5, bisect_iters))
                base = small_pool.tile([128, 1], dt.float32, tag="base")
                nc.vector.memset(base[:], -64.0)
                step = 1.0
                scr8 = gate_pool.tile([128, 16 * W16], dt.uint8)
                for _ in range(rounds):
                    cand = small_pool.tile([128, 1], dt.float32, tag="cand")
                    nc.vector.scalar_tensor_tensor(
                        cand[:], iota_pf[:], step, base[:],
                        op0=Alu.mult, op1=Alu.add)
                    part = small_pool.tile([128, 1], dt.float32, tag="part")
                    nc.vector.tensor_scalar(
                        scr8[:], z_bcast[:].rearrange("p a b -> p (a b)"),
                        cand[:, :1], None, op0=Alu.is_ge, op1=Alu.add,
                        accum_out=part[:])
                    geK = small_pool.tile([128, 1], dt.float32, tag="geK")
                    nc.vector.tensor_scalar(geK[:], part[:], kf, None,
                                            op0=Alu.is_ge)
                    js_ps = psum1_pool.tile([128, 1], dt.float32, tag="ps1")
                    nc.tensor.matmul(js_ps[:], ones128[:], geK[:],
                                     start=True, stop=True)
                    # base' = fl((j*-1)*step + base) — the SAME rounding
                    # path as the tested candidate, so the count(z >= base)
                    # >= K invariant holds bit-exactly.
                    jm1 = small_pool.tile([128, 1], dt.float32, tag="jm1")
                    nc.vector.tensor_scalar(jm1[:], js_ps[:], -1.0, None,
                                            op0=Alu.add)
                    nb = small_pool.tile([128, 1], dt.float32, tag="base")
                    nc.vector.scalar_tensor_tensor(
                        nb[:], jm1[:], step, base[:],
                        op0=Alu.mult, op1=Alu.add)
                    base = nb
                    step /= 128.0
                lo = base

                # ---- selection mask + compaction (wrapped-16 layout) ----
                sel16 = gate_pool.tile([16, W16], dt.uint8)
                nc.vector.tensor_scalar(sel16[:], z16[:], lo[:16, :1], None,
                                        op0=Alu.is_ge)
                # SG output tiles padded by 8 columns: fp32 score ties at
                # the threshold can make the selected count exceed K; the
                # overflow lands in the pad instead of the next tile.
                idneg = gate_pool.tile([16, W16], dt.float32)
                nc.vector.memset(idneg[:], -1.0)
                nc.vector.copy_predicated(idneg[:], sel16[:], idf16[:])
                idc = gate_pool.tile([16, K16 + 8], dt.float32)
                nf1 = gate_pool.tile([1, 1], dt.uint32)
                gneg = gate_pool.tile([16, W16], dt.float32)
                nc.vector.memset(gneg[:], -1.0)
                nc.vector.copy_predicated(gneg[:], sel16[:], sig16[:])
                gc = gate_pool.tile([16, K16 + 8], dt.float32)
                nf2 = gate_pool.tile([1, 1], dt.uint32)
                if not skip_sg:
                    nc.gpsimd.sparse_gather(idc[:], idneg[:],
                                            num_found=nf1[:])
                    sg2_inst = nc.gpsimd.sparse_gather(gc[:], gneg[:],
                                                       num_found=nf2[:])
                else:
                    fake = gate_pool.tile([16, K16], dt.int32)
                    nc.gpsimd.iota(fake[:], pattern=[[16, K16]], base=0,
                                   channel_multiplier=1)
                    nc.vector.tensor_copy(idc[:, :K16], fake[:])
                    nc.vector.memset(gc[:, :K16], 0.5)

                # idxs: fp32 -> i32 -> i16, replicated to all 8 core blocks
                idc_i32 = gate_pool.tile([16, K16], dt.int32)
                nc.vector.tensor_copy(idc_i32[:], idc[:, :K16])
                idc_i16 = gate_pool.tile([16, K16], dt.int16)
                nc.vector.tensor_copy(idc_i16[:], idc_i32[:])
                for b in range(8):
                    nc.sync.dma_start(idxs_tok[16 * b:16 * (b + 1), :],
                                      idc_i16[:])

                # gate values to per-partition [128, NCOLS] via DRAM bounce
                nc.sync.dma_start(
                    g_dram[:].rearrange("(c r) -> r c", r=16), gc[:, :K16])
                nc.sync.dma_start(
                    g_pp[:], g_dram[:].rearrange("(q p) -> p q", p=128))

            # ------- weights + dense zero fill (fill DMA idle time of the
            # gate phase; emitted after it so the gate loads win priority) ---
            w1_sb = w_pool.tile([128, HC, FF], dt.bfloat16)
            nc.sync.dma_start(
                w1_sb[:], W1_d[:].rearrange("(c p) f -> p c f", p=128))
            w2_sb = w_pool.tile([128, FC, H], dt.bfloat16)
            nc.sync.dma_start(
                w2_sb[:], W2_d[:].rearrange("(c p) h -> p c h", p=128))

            zero_bf = const_pool.tile([128, 2, H], dt.bfloat16)
            nc.vector.memset(zero_bf[:], 0.0)
            for i in range(N // 256):
                nc.sync.dma_start(
                    dense_d[256 * i:256 * (i + 1), :].rearrange(
                        "(c p) h -> p c h", p=128),
                    zero_bf[:])

            # ================= FFN phase ================
            with (
                tc.tile_pool(name="ex", bufs=2) as ex_pool,
                tc.tile_pool(name="hid", bufs=1) as hid_pool,
                tc.tile_pool(name="out", bufs=2) as out_pool,
            ):
                for g in range(NG if not skip_ffn else 0):
                    ex_T = ex_pool.tile([128, HC, TOKG], dt.bfloat16,
                                        tag="ex")
                    if skip_gather:
                        nc.vector.memset(ex_T[:], 0.01)
                    else:
                        g_inst = nc.gpsimd.dma_gather(
                            ex_T[:], x_bf[:],
                            idxs_tok[:, g * GCOLS:(g + 1) * GCOLS],
                            TOKG, TOKG, H, transpose=True)
                        if g == 0 and sg2_inst is not None:
                            _add_dep_helper(g_inst.ins, sg2_inst.ins,
                                            reason="keep sparse_gather lib "
                                                   "phase before mlp phase")

                    hid_sb = hid_pool.tile([128, FC, TOKG], dt.bfloat16,
                                           tag="hid")
                    for fc in range(FC):
                        ps1 = psum1_pool.tile([128, TOKG], dt.float32,
                                              tag="ps1")
                        for ci in range(HC):
                            nc.tensor.matmul(
                                ps1[:], w1_sb[:, ci, fc * 128:(fc + 1) * 128],
                                ex_T[:, ci, :],
                                start=(ci == 0), stop=(ci == HC - 1))
                        nc.scalar.activation(hid_sb[:, fc, :], ps1[:], act,
                                             bias=b1_pp[:, fc:fc + 1])

                    out_sb = out_pool.tile([128, SUBS, H], dt.bfloat16,
                                           tag="osb")
                    for s in range(SUBS):
                        col = g * SUBS + s
                        pso = psum2_pool.tile([128, H], dt.float32, tag="ps2")
                        for hh in range(0, H, NSTEP):
                            for fc in range(FC):
                                nc.tensor.matmul(
                                    pso[:, hh:hh + NSTEP],
                                    hid_sb[:, fc, s * 128:(s + 1) * 128],
                                    w2_sb[:, fc, hh:hh + NSTEP],
                                    start=(fc == 0), stop=(fc == FC - 1))
                        nc.vector.tensor_tensor(out_sb[:, s, :], pso[:],
                                                b2_bcast[:], op=Alu.add)
                        nc.vector.tensor_scalar(out_sb[:, s, :],
                                                out_sb[:, s, :],
                                                g_pp[:, col:col + 1], None,
                                                op0=Alu.mult)

                    if not skip_scatter:
                        nc.gpsimd.dma_scatter_add(
                            dense_d[:], out_sb[:],
                            idxs_tok[:, g * GCOLS:(g + 1) * GCOLS],
                            TOKG, TOKG, H)

            # ---------------- combine ----------------
            if skip_rs:
                nc.sync.dma_start(rs_out_d[:], dense_d[0:P, :])
            else:
                nc.gpsimd.collective_compute(
                    "ReduceScatter", Alu.add, replica_groups=groups,
                    ins=[dense_d[:]], outs=[rs_out_d[:]],
                )

            nc.sync.dma_start(y_d[:], rs_out_d[:])

    if do_compile:
        nc.compile()
        _split_excess_waits(nc)
    return nc


# ---------------------------------------------------------------------------
# host-side sharding + execution
# ---------------------------------------------------------------------------

def make_in_maps(x, Wg, W1, b1, W2, b2, N=8192, H=1024):
    xt = np.ascontiguousarray(x.reshape(N, H).astype(np.float32))
    x_bf = xt.astype(ml_dtypes.bfloat16)
    P = N // N_CORES
    in_maps = []
    for c in range(N_CORES):
        shard = xt[c * P:(c + 1) * P, :]
        in_maps.append({
            "xT_s": np.ascontiguousarray(shard.T),
            "x_bf": x_bf,
            "Wg": np.ascontiguousarray(Wg.astype(np.float32)),
            "W1": np.ascontiguousarray(W1[c].astype(ml_dtypes.bfloat16)),
            "W2": np.ascontiguousarray(W2[c].astype(ml_dtypes.bfloat16)),
            "b1": np.ascontiguousarray(b1[c].reshape(1, -1).astype(np.float32)),
            "b2": np.ascontiguousarray(b2[c].reshape(1, -1).astype(np.float32)),
        })
    return in_maps


_NC_CACHE = {}


def kernel(x, Wg, W1, b1, W2, b2):
    x = np.asarray(x)
    B, L, H = x.shape
    N = B * L
    FF = W1.shape[2]
    key = (N, H, FF)
    if key not in _NC_CACHE:
        _NC_CACHE[key] = build_moe_nc(N=N, H=H, FF=FF)
    nc = _NC_CACHE[key]
    in_maps = make_in_maps(np.asarray(x), np.asarray(Wg), np.asarray(W1),
                           np.asarray(b1), np.asarray(W2), np.asarray(b2),
                           N=N, H=H)
    from concourse.bass_utils import run_bass_kernel_spmd
    res = run_bass_kernel_spmd(nc, in_maps, core_ids=list(range(N_CORES)),
                               trace=False)
    out = np.concatenate([res.results[c]["y"] for c in range(N_CORES)], axis=0)
    return out.reshape(B, L, H).astype(np.float32)


# revision 26
# speedup vs baseline: 1.1967x; 1.1967x over previous
"""Expert-choice MoE kernel for 8 Trainium2 NeuronCores (Bacc/Tile).

Distribution: expert-parallel, one expert per core.
  - gate: each core computes fp32 scores z = x_shard @ Wg for its 1/8 token
    shard (pre-swizzled so the wrapped-16 view loads contiguously later),
    AllToAll -> each core holds the full (N,) score row of ITS expert.
  - top-k (k=2048 of N=8192): exact fp32 threshold via a parallel-128
    candidate search (PE row-broadcast of the scores + one
    tensor_scalar/accum count per round, 5 rounds = 128^5 resolution),
    then index compaction with the gpsimd sparse_gather ucode kernel
    (output padded by 8 columns to absorb score ties at the threshold).
  - dispatch: ONE dma_gather(transpose=True) per 512-token group pulls the
    selected rows from HBM already transposed to [h, tok] bf16 layout.
  - expert FFN in bf16 (fp32 accumulation), erf-Gelu on the scalar engine,
    fp32 gate multiply on the bf16 output.
  - combine: ONE dma_scatter_add (SDMA CCE add) per group into a zeroed
    bf16 (N, H) dense buffer, ReduceScatter (add, bf16) across the 8
    cores; y is emitted bf16 and upcast on the host.

Built on Bacc (not raw Bass): Bacc.compile() runs insert_library_loads
and codegen_inst_isa_subclasses, which this walrus build needs to accept
the sparse_gather/dma_gather/dma_scatter_add Pool-ucode instructions.
"""

import sys

for _p in ("/opt/trn_rl_repo",):
    if _p not in sys.path:
        sys.path.insert(0, _p)

import numpy as np
import ml_dtypes

import concourse.bass as bass
import concourse.bacc as bacc
import concourse.mybir as mybir
import concourse.tile as tile
from concourse.bass import _add_dep_helper

# ---------------------------------------------------------------------------
# Patch: this walrus build rejects >1 sync-wait on the SP Drain that
# TileContext emits at kernel exit. Split the global-clock waits across
# several drains (1 wait each).
# ---------------------------------------------------------------------------
from concourse.vector_clock import ScopedClock

_MAX_DRAIN_WAITS = 1


def _patched_drain_and_barrier(self, tick_clock, wait_clock):
    nc = self.nc
    probe = nc.sync.drain()
    wait_clock.add_sem_waits(probe.ins, ScopedClock({None: tick_clock.global_clock}))
    si = probe.ins.sync_info
    waits = list(si.on_wait or []) if si is not None else []
    if len(waits) > _MAX_DRAIN_WAITS:
        probe.ins.sync_info = mybir.SyncInfo(
            on_wait=waits[:_MAX_DRAIN_WAITS],
            on_update=list(si.on_update or []),
        )
        for i in range(_MAX_DRAIN_WAITS, len(waits), _MAX_DRAIN_WAITS):
            extra = nc.sync.drain()
            extra.ins.sync_info = mybir.SyncInfo(
                on_wait=waits[i : i + _MAX_DRAIN_WAITS], on_update=[]
            )
    nc.all_engine_barrier()
    assert self.sems is not None
    popped = nc._tile_sem_poison_stack.pop()
    assert popped is self._sem_poison
    nc.clear_and_free_semaphores(list(self.sems.allocated().values()))
    nc.all_engine_barrier()


tile.TileContext._drain_and_barrier = _patched_drain_and_barrier

_WSPLIT_LIMIT = 1
_wsplit_ctr = [0]


def _split_excess_waits(nc, limit=_WSPLIT_LIMIT):
    """This walrus build encodes at most `limit` sync-wait commands per
    instruction; hoist excess waits onto same-engine Drain instructions
    inserted immediately before (per-engine streams execute in order)."""
    f = nc.m.functions[0]
    for b in f.blocks:
        insts = b.instructions
        out = []
        changed = False
        for inst in insts:
            si = getattr(inst, "sync_info", None)
            waits = list(si.on_wait or []) if si is not None else []
            eng = getattr(inst, "engine", None)
            if len(waits) > limit and eng is not None and \
                    eng != mybir.EngineType.Unassigned:
                keep = waits[-limit:]
                extra = waits[:-limit]
                for i in range(0, len(extra), limit):
                    d = mybir.InstDrain(
                        name=f"WSPLIT-{_wsplit_ctr[0]}", ins=[], outs=[])
                    _wsplit_ctr[0] += 1
                    d.engine = eng
                    d.sync_info = mybir.SyncInfo(
                        on_wait=extra[i:i + limit], on_update=[])
                    out.append(d)
                    nc.register_instruction(d, overwrite=True)
                inst.sync_info = mybir.SyncInfo(
                    on_wait=keep, on_update=list(si.on_update or []))
                changed = True
            out.append(inst)
        if changed:
            b.instructions = out

dt = mybir.dt
Alu = mybir.AluOpType
Act = mybir.ActivationFunctionType

N_CORES = 8

FULL = dict(N=8192, H=1024, FF=4096, E=8, K=2048)


def build_moe_nc(N=8192, H=1024, FF=4096, E=8, K=2048, TOKG=512, act=None,
                 do_compile=True, bisect_iters=33, skip_sg=False,
                 skip_ffn=False, skip_rs=False, skip_gather=False,
                 skip_scatter=False):
    """Build the SPMD Bacc program (same program on all 8 cores)."""
    assert E == N_CORES
    P = N // N_CORES          # tokens per shard
    HC = H // 128             # h chunks
    FC = FF // 128            # ff chunks
    NG = K // TOKG            # token groups
    SUBS = TOKG // 128        # 128-token subtiles per group
    NCOLS = K // 128          # compact cols in [128, NCOLS] layout
    ZF = N // 128             # free size of the [128, ZF] score layout
    W16 = N // 16             # free size of the [16, W16] wrapped layout
    K16 = K // 16             # compact cols in [16, K16] wrapped layout
    GCOLS = TOKG // 16        # idx cols consumed per group
    assert K % TOKG == 0 and TOKG % 128 == 0 and P % 128 == 0
    assert K16 <= 512  # sparse_gather output limit
    if act is None:
        act = Act.Gelu
    NSTEP = min(512, H)

    nc = bacc.Bacc(None, target_bir_lowering=False, debug=False,
                   num_devices=N_CORES)

    # ---- I/O ----
    xT_s = nc.dram_tensor("xT_s", [H, P], dt.float32, kind="ExternalInput")
    x_bf = nc.dram_tensor("x_bf", [N, H], dt.bfloat16, kind="ExternalInput")
    Wg_d = nc.dram_tensor("Wg", [H, E], dt.float32, kind="ExternalInput")
    W1_d = nc.dram_tensor("W1", [H, FF], dt.bfloat16, kind="ExternalInput")
    W2_d = nc.dram_tensor("W2", [FF, H], dt.bfloat16, kind="ExternalInput")
    b1_d = nc.dram_tensor("b1", [1, FF], dt.float32, kind="ExternalInput")
    b2_d = nc.dram_tensor("b2", [1, H], dt.float32, kind="ExternalInput")
    # y is emitted in bf16 (the combine is bf16 anyway); the host upcasts
    y_d = nc.dram_tensor("y", [P, H], dt.bfloat16, kind="ExternalOutput")

    # ---- internal DRAM ----
    z_loc_d = nc.dram_tensor("z_loc", [E, P], dt.float32)
    z_e_d = nc.dram_tensor("z_e", [N_CORES, P], dt.float32)
    g_dram = nc.dram_tensor("g_dram", [K], dt.float32)
    dense_d = nc.dram_tensor("dense", [N, H], dt.bfloat16)
    rs_out_d = nc.dram_tensor("rs_out", [P, H], dt.bfloat16)

    groups = [list(range(N_CORES))]

    with tile.TileContext(nc) as tc:
        with (
            tc.tile_pool(name="const", bufs=1) as const_pool,
            tc.tile_pool(name="w", bufs=1) as w_pool,
            tc.tile_pool(name="psum1", bufs=2, space="PSUM") as psum1_pool,
            tc.tile_pool(name="psum2", bufs=2, space="PSUM") as psum2_pool,
        ):
            # ---------------- persistent constants ----------------
            ones1 = const_pool.tile([1, 128], dt.float32)
            nc.vector.memset(ones1[:], 1.0)
            ones128 = const_pool.tile([128, 128], dt.float32)
            nc.vector.memset(ones128[:], 1.0)

            # b2 broadcast [128, H] (constant along tokens)
            b2_sb = const_pool.tile([1, H], dt.float32)
            nc.sync.dma_start(b2_sb[:], b2_d[:])
            b2_ps = psum2_pool.tile([128, H], dt.float32, tag="ps2")
            for hh in range(0, H, NSTEP):
                nc.tensor.matmul(b2_ps[:, hh:hh + NSTEP], ones1[:],
                                 b2_sb[:, hh:hh + NSTEP], start=True, stop=True)
            b2_bcast = const_pool.tile([128, H], dt.float32)
            nc.vector.tensor_copy(b2_bcast[:], b2_ps[:])

            # b1 per-partition [128, FC]
            b1_pp = const_pool.tile([128, FC], dt.float32)
            nc.sync.dma_start(
                b1_pp[:], b1_d[:].rearrange("o (c p) -> (o p) c", p=128))

            # persistent routing outputs (filled by the gate phase)
            idxs_tok = const_pool.tile([128, K16], dt.int16)
            g_pp = const_pool.tile([128, NCOLS], dt.float32)

            # ================= gate phase (scoped pool) ================
            # Emitted BEFORE the (much larger) weight/zero-fill DMAs so the
            # scheduler gives the latency-critical gate inputs DMA priority.
            sg2_inst = None
            with (
                tc.tile_pool(name="gate", bufs=1) as gate_pool,
                tc.tile_pool(name="small", bufs=2) as small_pool,
            ):
                xT_sb = gate_pool.tile([128, HC, P], dt.float32)
                nc.sync.dma_start(
                    xT_sb[:], xT_s[:].rearrange("(c p) t -> p c t", p=128))
                wg_sb = gate_pool.tile([128, HC, E], dt.float32)
                nc.sync.dma_start(
                    wg_sb[:], Wg_d[:].rearrange("(c p) e -> p c e", p=128))

                # z_sb_loc rows are written PRE-SWIZZLED (token u stored at
                # column (u%16)*64 + u//16) so that after the AllToAll the
                # wrapped-16 [16, W16] view is a contiguous-stride load.
                z_sb_loc = gate_pool.tile([E, P], dt.float32)
                z_loc_sw = z_sb_loc[:].rearrange("e (r w) -> e w r", r=16)
                for t0 in range(0, P, 512):
                    zw = min(512, P - t0)
                    z_ps = psum1_pool.tile([E, 512], dt.float32, tag="ps1")
                    for ci in range(HC):
                        nc.tensor.matmul(z_ps[:, :zw], wg_sb[:, ci, :],
                                         xT_sb[:, ci, t0:t0 + zw],
                                         start=(ci == 0), stop=(ci == HC - 1))
                    nc.vector.tensor_copy(
                        z_loc_sw[:, t0 // 16:(t0 + zw) // 16, :],
                        z_ps[:, :zw])
                nc.sync.dma_start(z_loc_d[:], z_sb_loc[:])

                # core c receives every shard's scores for expert c
                nc.gpsimd.collective_compute(
                    "AllToAll", Alu.bypass, replica_groups=groups,
                    ins=[z_loc_d[:]], outs=[z_e_d[:]],
                )

                # wrapped-16 view (token j at [j%16, j//16]); contiguous
                # 64-element runs thanks to the sender-side swizzle
                # (slot shared with the now-dead z_sb_loc)
                z16 = gate_pool.tile([16, W16], dt.float32, tag="z_sb_loc")
                nc.sync.dma_start(
                    z16[:].rearrange("r (q w) -> r q w", q=E),
                    z_e_d[:].rearrange("q (r w) -> r q w", r=16))

                # compaction inputs that do not depend on the threshold —
                # emitted first so they overlap the A2A / bisection
                ids16 = gate_pool.tile([16, W16], dt.int32)
                nc.gpsimd.iota(ids16[:], pattern=[[16, W16]], base=0,
                               channel_multiplier=1)
                idf16 = gate_pool.tile([16, W16], dt.float32)
                nc.vector.tensor_copy(idf16[:], ids16[:])
                sig16 = gate_pool.tile([16, W16], dt.float32)
                nc.scalar.activation(sig16[:], z16[:], Act.Sigmoid)


                # ---- parallel-128 search for the k-th largest score ----
                # Every partition holds ALL N scores (PE row-broadcast of
                # z16); round r tests the 128 candidates base + p*step_r at
                # once (one tensor_scalar with accum_out gives, for every
                # partition p, the count of scores >= its candidate).
                # base' = base + (j*-1)*step with j* = #candidates whose
                # count >= K keeps count(z >= base) >= K invariant; each
                # round shrinks the bracket 128x. 5 rounds -> 3.7e-9, below
                # fp32 ulp of the scores.
                # eqm[k', k*128+m] = (k==k'): selector for the broadcast
                # int iota shares the (later-used) scr8 slot — same 8KB
                eqm_i = gate_pool.tile([16, 16 * 128], dt.int32, tag="scr8")
                nc.gpsimd.iota(eqm_i[:], pattern=[[1, 16], [0, 128]], base=0,
                               channel_multiplier=-1)
                eqm = gate_pool.tile([16, 16 * 128], dt.float32)
                nc.vector.tensor_scalar(eqm[:], eqm_i[:], 0, None,
                                        op0=Alu.is_equal)
                iota_p = gate_pool.tile([128, 1], dt.int32)
                nc.gpsimd.iota(iota_p[:], pattern=[[1, 1]], base=0,
                               channel_multiplier=1)
                iota_pf = gate_pool.tile([128, 1], dt.float32)
                nc.vector.tensor_copy(iota_pf[:], iota_p[:])

                z_bcast = gate_pool.tile([128, 16, W16], dt.float32,
                                         tag="xT_sb")  # reuse the xT slot
                for k in range(16):
                    for w0 in range(0, W16, 512):
                        ww = min(512, W16 - w0)
                        zb_ps = psum1_pool.tile([128, 512], dt.float32,
                                                tag="ps1")
                        nc.tensor.matmul(zb_ps[:, :ww],
                                         eqm[:, k * 128:(k + 1) * 128],
                                         z16[:, w0:w0 + ww],
                                         start=True, stop=True)
                        nc.vector.tensor_copy(z_bcast[:, k, w0:w0 + ww],
                                              zb_ps[:, :ww])

                kf = float(K)
                rounds = max(1, min(5, bisect_iters))
                base = small_pool.tile([128, 1], dt.float32, tag="base")
                nc.vector.memset(base[:], -64.0)
                step = 1.0
                scr8 = gate_pool.tile([128, 16 * W16], dt.uint8)
                for _ in range(rounds):
                    cand = small_pool.tile([128, 1], dt.float32, tag="cand")
                    nc.vector.scalar_tensor_tensor(
                        cand[:], iota_pf[:], step, base[:],
                        op0=Alu.mult, op1=Alu.add)
                    part = small_pool.tile([128, 1], dt.float32, tag="part")
                    nc.vector.tensor_scalar(
                        scr8[:], z_bcast[:].rearrange("p a b -> p (a b)"),
                        cand[:, :1], None, op0=Alu.is_ge, op1=Alu.add,
                        accum_out=part[:])
                    geK = small_pool.tile([128, 1], dt.float32, tag="geK")
                    nc.vector.tensor_scalar(geK[:], part[:], kf, None,
                                            op0=Alu.is_ge)
                    js_ps = psum1_pool.tile([128, 1], dt.float32, tag="ps1")
                    nc.tensor.matmul(js_ps[:], ones128[:], geK[:],
                                     start=True, stop=True)
                    # base' = fl((j*-1)*step + base) — the SAME rounding
                    # path as the tested candidate, so the count(z >= base)
                    # >= K invariant holds bit-exactly.
                    jm1 = small_pool.tile([128, 1], dt.float32, tag="jm1")
                    nc.vector.tensor_scalar(jm1[:], js_ps[:], -1.0, None,
                                            op0=Alu.add)
                    nb = small_pool.tile([128, 1], dt.float32, tag="base")
                    nc.vector.scalar_tensor_tensor(
                        nb[:], jm1[:], step, base[:],
                        op0=Alu.mult, op1=Alu.add)
                    base = nb
                    step /= 128.0
                lo = base

                # ---- selection mask + compaction (wrapped-16 layout) ----
                sel16 = gate_pool.tile([16, W16], dt.uint8)
                nc.vector.tensor_scalar(sel16[:], z16[:], lo[:16, :1], None,
                                        op0=Alu.is_ge)
                # SG output tiles padded by 8 columns: fp32 score ties at
                # the threshold can make the selected count exceed K; the
                # overflow lands in the pad instead of the next tile.
                idneg = gate_pool.tile([16, W16], dt.float32)
                nc.vector.memset(idneg[:], -1.0)
                nc.vector.copy_predicated(idneg[:], sel16[:], idf16[:])
                idc = gate_pool.tile([16, K16 + 8], dt.float32)
                nf1 = gate_pool.tile([1, 1], dt.uint32)
                gneg = gate_pool.tile([16, W16], dt.float32)
                nc.vector.memset(gneg[:], -1.0)
                nc.vector.copy_predicated(gneg[:], sel16[:], sig16[:])
                gc = gate_pool.tile([16, K16 + 8], dt.float32)
                nf2 = gate_pool.tile([1, 1], dt.uint32)
                if not skip_sg:
                    nc.gpsimd.sparse_gather(idc[:], idneg[:],
                                            num_found=nf1[:])
                    sg2_inst = nc.gpsimd.sparse_gather(gc[:], gneg[:],
                                                       num_found=nf2[:])
                else:
                    fake = gate_pool.tile([16, K16], dt.int32)
                    nc.gpsimd.iota(fake[:], pattern=[[16, K16]], base=0,
                                   channel_multiplier=1)
                    nc.vector.tensor_copy(idc[:, :K16], fake[:])
                    nc.vector.memset(gc[:, :K16], 0.5)

                # idxs: fp32 -> i32 -> i16, replicated to all 8 core blocks
                idc_i32 = gate_pool.tile([16, K16], dt.int32)
                nc.vector.tensor_copy(idc_i32[:], idc[:, :K16])
                idc_i16 = gate_pool.tile([16, K16], dt.int16)
                nc.vector.tensor_copy(idc_i16[:], idc_i32[:])
                for b in range(8):
                    nc.sync.dma_start(idxs_tok[16 * b:16 * (b + 1), :],
                                      idc_i16[:])

                # gate values to per-partition [128, NCOLS] via DRAM bounce
                nc.sync.dma_start(
                    g_dram[:].rearrange("(c r) -> r c", r=16), gc[:, :K16])
                nc.sync.dma_start(
                    g_pp[:], g_dram[:].rearrange("(q p) -> p q", p=128))

            # ------- weights + dense zero fill (fill DMA idle time of the
            # gate phase; emitted after it so the gate loads win priority) ---
            w1_sb = w_pool.tile([128, HC, FF], dt.bfloat16)
            nc.sync.dma_start(
                w1_sb[:], W1_d[:].rearrange("(c p) f -> p c f", p=128))
            w2_sb = w_pool.tile([128, FC, H], dt.bfloat16)
            nc.sync.dma_start(
                w2_sb[:], W2_d[:].rearrange("(c p) h -> p c h", p=128))

            zero_bf = const_pool.tile([128, 2, H], dt.bfloat16)
            nc.vector.memset(zero_bf[:], 0.0)
            for i in range(N // 256):
                nc.sync.dma_start(
                    dense_d[256 * i:256 * (i + 1), :].rearrange(
                        "(c p) h -> p c h", p=128),
                    zero_bf[:])

            # ================= FFN phase ================
            with (
                tc.tile_pool(name="ex", bufs=2) as ex_pool,
                tc.tile_pool(name="hid", bufs=1) as hid_pool,
                tc.tile_pool(name="out", bufs=2) as out_pool,
            ):
                for g in range(NG if not skip_ffn else 0):
                    ex_T = ex_pool.tile([128, HC, TOKG], dt.bfloat16,
                                        tag="ex")
                    if skip_gather:
                        nc.vector.memset(ex_T[:], 0.01)
                    else:
                        g_inst = nc.gpsimd.dma_gather(
                            ex_T[:], x_bf[:],
                            idxs_tok[:, g * GCOLS:(g + 1) * GCOLS],
                            TOKG, TOKG, H, transpose=True)
                        if g == 0 and sg2_inst is not None:
                            _add_dep_helper(g_inst.ins, sg2_inst.ins,
                                            reason="keep sparse_gather lib "
                                                   "phase before mlp phase")

                    hid_sb = hid_pool.tile([128, FC, TOKG], dt.bfloat16,
                                           tag="hid")
                    for fc in range(FC):
                        ps1 = psum1_pool.tile([128, TOKG], dt.float32,
                                              tag="ps1")
                        for ci in range(HC):
                            nc.tensor.matmul(
                                ps1[:], w1_sb[:, ci, fc * 128:(fc + 1) * 128],
                                ex_T[:, ci, :],
                                start=(ci == 0), stop=(ci == HC - 1))
                        nc.scalar.activation(hid_sb[:, fc, :], ps1[:], act,
                                             bias=b1_pp[:, fc:fc + 1])

                    out_sb = out_pool.tile([128, SUBS, H], dt.bfloat16,
                                           tag="osb")
                    for s in range(SUBS):
                        col = g * SUBS + s
                        pso = psum2_pool.tile([128, H], dt.float32, tag="ps2")
                        for hh in range(0, H, NSTEP):
                            for fc in range(FC):
                                nc.tensor.matmul(
                                    pso[:, hh:hh + NSTEP],
                                    hid_sb[:, fc, s * 128:(s + 1) * 128],
                                    w2_sb[:, fc, hh:hh + NSTEP],
                                    start=(fc == 0), stop=(fc == FC - 1))
                        nc.vector.tensor_tensor(out_sb[:, s, :], pso[:],
                                                b2_bcast[:], op=Alu.add)
                        nc.vector.tensor_scalar(out_sb[:, s, :],
                                                out_sb[:, s, :],
                                                g_pp[:, col:col + 1], None,
                                                op0=Alu.mult)

                    if not skip_scatter:
                        nc.gpsimd.dma_scatter_add(
                            dense_d[:], out_sb[:],
                            idxs_tok[:, g * GCOLS:(g + 1) * GCOLS],
                            TOKG, TOKG, H)

            # ---------------- combine ----------------
            if skip_rs:
                nc.sync.dma_start(rs_out_d[:], dense_d[0:P, :])
            else:
                nc.gpsimd.collective_compute(
                    "ReduceScatter", Alu.add, replica_groups=groups,
                    ins=[dense_d[:]], outs=[rs_out_d[:]],
                )

            nc.sync.dma_start(y_d[:], rs_out_d[:])

    if do_compile:
        nc.compile()
        _split_excess_waits(nc)
    return nc


# ---------------------------------------------------------------------------
# host-side sharding + execution
# ---------------------------------------------------------------------------

def make_in_maps(x, Wg, W1, b1, W2, b2, N=8192, H=1024):
    xt = np.ascontiguousarray(x.reshape(N, H).astype(np.float32))
    x_bf = xt.astype(ml_dtypes.bfloat16)
    P = N // N_CORES
    in_maps = []
    for c in range(N_CORES):
        shard = xt[c * P:(c + 1) * P, :]
        in_maps.append({
            "xT_s": np.ascontiguousarray(shard.T),
            "x_bf": x_bf,
            "Wg": np.ascontiguousarray(Wg.astype(np.float32)),
            "W1": np.ascontiguousarray(W1[c].astype(ml_dtypes.bfloat16)),
            "W2": np.ascontiguousarray(W2[c].astype(ml_dtypes.bfloat16)),
            "b1": np.ascontiguousarray(b1[c].reshape(1, -1).astype(np.float32)),
            "b2": np.ascontiguousarray(b2[c].reshape(1, -1).astype(np.float32)),
        })
    return in_maps


_NC_CACHE = {}


def kernel(x, Wg, W1, b1, W2, b2):
    x = np.asarray(x)
    B, L, H = x.shape
    N = B * L
    FF = W1.shape[2]
    key = (N, H, FF)
    if key not in _NC_CACHE:
        _NC_CACHE[key] = build_moe_nc(N=N, H=H, FF=FF)
    nc = _NC_CACHE[key]
    in_maps = make_in_maps(np.asarray(x), np.asarray(Wg), np.asarray(W1),
                           np.asarray(b1), np.asarray(W2), np.asarray(b2),
                           N=N, H=H)
    from concourse.bass_utils import run_bass_kernel_spmd
    res = run_bass_kernel_spmd(nc, in_maps, core_ids=list(range(N_CORES)),
                               trace=False)
    out = np.concatenate([res.results[c]["y"] for c in range(N_CORES)], axis=0)
    return out.reshape(B, L, H).astype(np.float32)
